# revision 3
# baseline (speedup 1.0000x reference)
"""Trainium2 Bass kernel: Conv2d(1->64, k=7, valid) on data [32,1,224,224] f32.

Strategy: data-parallel over batch (4 images per core on 8 cores).
Per core: im2col matmul in fp16 (K=49 taps, M=64 out-channels, N=spatial),
PSUM fp32 accumulate, fp32 output.

Data movement:
  - host: cast data to fp16, cut each image into 8 row-blocks of 34 rows
    (28 output rows + 6 halo), 2 copies of each block for DMA read
    parallelism; blocks land on 32 SBUF partitions spread over all 16 AXI
    ports.
  - im2col expansion: SBUF->SBUF SWDGE DMAs; dst tile [49, 28*224] fp16,
    src = one slab partition, 49 fully-contiguous descriptors with
    overlapping reads (ky -> row offset, kx -> element offset).
  - matmul: lhsT = W^T [49, 64] fp16 (constant), rhs = im2col slice
    [49, 448] (2 output rows of 224), out -> PSUM [*, 448] f32.  Pairs of
    matmuls write partitions 0-63 / 64-127 (col tile_position) so the
    PSUM->SBUF copy runs full 128-partition width on DVE/ACT.
  - output: [64ch, 14 rows * 224] fp32 halves DMA'd HBM-ward on the two
    HWDGE rings (sync + scalar).  Cols 218..223 are garbage (kx wraps into
    next row) and get sliced off on the host, as do rows >= 218.
"""

import numpy as np

# ---- problem constants (hardcoded; kernel.py must be self-contained) ----
B = 32            # full batch
OC = 64           # out channels
KS = 7            # kernel size
H = 224           # input H=W
OH = 218          # valid output rows/cols
OW = 224          # computed output width (incl 6 garbage cols)
NCORES = 8
IPC = B // NCORES  # images per core = 4

BLK = 28          # output rows per block
NBLK = 8          # row blocks per image (8*28 = 224 >= 218)
SRC_ROWS = BLK + KS - 1  # 34
SLAB = SRC_ROWS * H + 8  # fp16 elements per slab (pad 8 elems = 16B)
NTILES = IPC * NBLK      # 32 im2col tiles per core
NCOLS = BLK * OW         # 6272 im2col columns per tile
NMM = NCOLS // 448       # 14 matmuls per tile (448 cols = 2 rows each)

# slab placement: slab s -> partition PL32[s % 32], free slot s // 32
# partitions chosen so all 16 SBUF AXI ports are covered.
RUNBASE = [0, 64, 1, 65]
PL32 = [rb + 4 * i for rb in RUNBASE for i in range(8)]

_CACHE = {}


def _build():
    import concourse.bass as bass
    import concourse.mybir as mybir
    import concourse.tile as tile
    from concourse import bacc

    nc = bacc.Bacc("TRN2", target_bir_lowering=False, debug=False)

    xb = nc.dram_tensor("xb", [2, 4, 8, SLAB], mybir.dt.float16,
                        kind="ExternalInput")
    wT = nc.dram_tensor("wT", [KS * KS, OC], mybir.dt.float16,
                        kind="ExternalInput")
    out = nc.dram_tensor("out", [IPC, OC, OH, OW], mybir.dt.float32,
                         kind="ExternalOutput")

    with tile.TileContext(nc) as tc:
        with (
            tc.tile_pool(name="src", bufs=1) as src_pool,
            tc.tile_pool(name="wp", bufs=1) as w_pool,
            tc.tile_pool(name="i2c", bufs=6) as i2c_pool,
            tc.tile_pool(name="ob", bufs=4) as ob_pool,
            tc.tile_pool(name="ps", bufs=6, space="PSUM") as ps_pool,
        ):
            # persistent tiles
            srct = src_pool.tile([128, 2 * SLAB], mybir.dt.float16)
            wt = w_pool.tile([KS * KS, OC], mybir.dt.float16)

            p_stride = srct.ap[0][0]  # partition pitch in elements

            nc.sync.dma_start(out=wt[:, :], in_=wT[:, :])

            # input load: 8 DMAs, each 8 partitions stride 4
            for slot in range(2):
                for run in range(4):
                    dst = bass.AP(
                        tensor=srct.tensor,
                        offset=srct.offset + RUNBASE[run] * p_stride
                        + slot * SLAB,
                        ap=[[4 * p_stride, 8], [1, SLAB]],
                    )
                    nc.gpsimd.dma_start(out=dst, in_=xb[slot, run, :, :])

            def slab_loc(s):
                idx = s % 32
                return PL32[idx], (s // 32) * SLAB

            for t in range(NTILES):
                # t = img * NBLK + blk
                img, blk = divmod(t, NBLK)
                i2c = i2c_pool.tile([KS * KS, NCOLS], mybir.dt.float16,
                                    tag="i2c")
                # im2col fill: one DMA per ky (DMA APs are limited to 3
                # dims).  ky 0..3 read source copy A, ky 4..6 copy B =>
                # 2 AXI source ports per tile.
                for ky in range(KS):
                    p, off = slab_loc(2 * t + (ky >= 4))
                    src = bass.AP(
                        tensor=srct.tensor,
                        offset=srct.offset + p * p_stride + off + ky * H,
                        ap=[[p_stride, 1], [1, KS], [1, NCOLS]],
                    )
                    nc.gpsimd.dma_start(
                        out=i2c[ky * KS:(ky + 1) * KS, :], in_=src)

                ob = ob_pool.tile([128, NMM // 2 * 448], mybir.dt.float32,
                                  tag="ob")
                for j in range(NMM // 2):
                    ps = ps_pool.tile([128, 448], mybir.dt.float32, tag="ps")
                    # half 0: output rows 2j, 2j+1 -> psum partitions 0:64
                    nc.tensor.matmul(
                        ps[0:OC, :], wt[:, :],
                        i2c[:, 448 * j: 448 * (j + 1)],
                        start=True, stop=True)
                    # half 1: output rows 14+2j, 14+2j+1 -> partitions 64:128
                    nc.tensor.matmul(
                        ps[OC:128, :], wt[:, :],
                        i2c[:, 14 * OW + 448 * j: 14 * OW + 448 * (j + 1)],
                        start=True, stop=True)
                    if j % 2 == 0:
                        nc.vector.tensor_copy(
                            ob[:, 448 * j: 448 * (j + 1)], ps[:, :])
                    else:
                        nc.scalar.copy(
                            ob[:, 448 * j: 448 * (j + 1)], ps[:, :])

                # output DMAs: half h = rows [28*blk + 14h, +14) of image img
                for h, eng in ((0, nc.sync), (1, nc.scalar)):
                    r0 = BLK * blk + 14 * h
                    nrows = min(14, OH - r0)
                    if nrows <= 0:
                        continue
                    eng.dma_start(
                        out=out[img, :, r0: r0 + nrows, :],
                        in_=ob[64 * h: 64 * h + OC, : nrows * OW])

    nc.compile()
    return nc


def _prep_inputs(data, weight):
    """Host-side prep: fp16 cast + slab layout per core."""
    d16 = np.ascontiguousarray(data.reshape(B, H, H)).astype(np.float16)
    # pad rows so block 7 (rows 196..229) exists
    dpad = np.zeros((B, NBLK * BLK + KS - 1 + 2, H), dtype=np.float16)
    dpad[:, :H, :] = d16
    wt = np.ascontiguousarray(
        weight.reshape(OC, KS * KS).T).astype(np.float16)

    in_maps = []
    for c in range(NCORES):
        xb = np.zeros((2, 4, 8, SLAB), dtype=np.float16)
        for img in range(IPC):
            gimg = c * IPC + img
            for blk in range(NBLK):
                flat = dpad[gimg, BLK * blk: BLK * blk + SRC_ROWS, :].ravel()
                for half in range(2):
                    s = 2 * (img * NBLK + blk) + half
                    idx = s % 32
                    slot, run, i = s // 32, idx // 8, idx % 8
                    xb[slot, run, i, : SRC_ROWS * H] = flat
        in_maps.append({"xb": xb, "wT": wt})
    return in_maps


def kernel(data, weight):
    from concourse.bass_utils import run_bass_kernel_spmd

    if "nc" not in _CACHE:
        _CACHE["nc"] = _build()
    nc = _CACHE["nc"]

    in_maps = _prep_inputs(np.asarray(data), np.asarray(weight))
    res = run_bass_kernel_spmd(nc, in_maps, core_ids=list(range(NCORES)))
    outs = [r["out"] for r in res.results]
    full = np.concatenate(outs, axis=0)  # [32, 64, 218, 224]
    return np.ascontiguousarray(full[:, :, :, :OH]).astype(np.float32)


# revision 4
# speedup vs baseline: 1.6112x; 1.6112x over previous
"""Trainium2 Bass kernel: Conv2d(1->64, k=7, valid) on data [32,1,224,224] f32.

Data-parallel over batch (4 images per core on 8 cores).  Per core:
im2col matmul in fp16 (K=49 taps, M=64 out-channels), PSUM fp32, fp32 out.

Layout/pipeline (per core, 32 row-block "tiles" of 28 output rows):
  - host: fp16 cast; for each tile, SEVEN copies of its 34-row source
    block, copy ky pre-shifted down by ky rows.  Copies of one tile sit at
    partitions base+4*ky (7 distinct AXI ports); even tiles use the lower
    partition half / even ports, odd tiles the upper half / odd ports.
  - im2col: ONE SWDGE DMA per tile (3-dim AP): src dim0 walks the 7 slab
    copies, dim1 the 7 kx shifts (overlapping reads), dim2 a contiguous
    28*224-col run.  dst = [49, 6272] fp16 at partition base 0 (even
    tiles) or 64 (odd tiles).
  - matmul: pairs (even tile, odd tile): lhsT = W^T [49,64] fp16 at row
    base 0/64, out -> psum[0:64]/[64:128] of one bank.  Alternating row
    groups lets LDWEIGHTS overlap in-flight matmuls.
  - copy: psum [128,448] -> ob tile full width, DVE/ACT alternating.
  - out: one DMA per tile [64ch, 28*224 f32]; even tiles on the sync
    HWDGE ring, odd on scalar.  Cols 218..223 are garbage (kx wrap) and
    are sliced off on the host, as are rows >= 218.
"""

import numpy as np

B = 32            # full batch
OC = 64           # out channels
KS = 7            # kernel size
H = 224           # input H=W
OH = 218          # valid output rows/cols
OW = 224          # computed output width (incl 6 garbage cols)
NCORES = 8
IPC = B // NCORES  # images per core

BLK = 28          # output rows per tile
NBLK = 8          # tiles per image
SRC_ROWS = 34     # rows stored per slab copy
SLAB = SRC_ROWS * H + 8   # 7624 fp16 elements per slab
NTILES = IPC * NBLK       # 32 tiles per core
NPAIRS = NTILES // 2
NCOLS = BLK * OW          # 6272 im2col columns per tile
NMM = NCOLS // 448        # 14 matmuls per tile

# slab-group bases: tile t -> 7 slabs at partitions base+4*ky, where
# base = (64 if t odd) + BASES[(t//2) % 8], free slot (t//2) // 8.
BASES = [0, 1, 2, 3, 28, 29, 30, 31]

_CACHE = {}


def _tile_src(t):
    q = t // 2
    base = BASES[q % 8] + (64 if (t % 2) else 0)
    return base, q // 8  # partition base, slot


def _build():
    import concourse.bass as bass
    import concourse.mybir as mybir
    import concourse.tile as tile
    from concourse import bacc

    nc = bacc.Bacc("TRN2", target_bir_lowering=False, debug=False)

    xb = nc.dram_tensor("xb", [2, 128, SLAB], mybir.dt.float16,
                        kind="ExternalInput")
    wT = nc.dram_tensor("wT", [KS * KS, OC], mybir.dt.float16,
                        kind="ExternalInput")
    out = nc.dram_tensor("out", [IPC, OC, OH, OW], mybir.dt.float32,
                         kind="ExternalOutput")

    with tile.TileContext(nc) as tc:
        with (
            tc.tile_pool(name="src", bufs=1) as src_pool,
            tc.tile_pool(name="wp", bufs=1) as w_pool,
            tc.tile_pool(name="i2c", bufs=3) as i2c_pool,
            tc.tile_pool(name="ob", bufs=3) as ob_pool,
            tc.tile_pool(name="ps", bufs=8, space="PSUM") as ps_pool,
        ):
            srct = src_pool.tile([128, 2 * SLAB], mybir.dt.float16)
            wt = w_pool.tile([128, OC], mybir.dt.float16)

            p_stride = srct.ap[0][0]  # partition pitch in elements

            nc.gpsimd.dma_start(out=wt[0:49, :], in_=wT[:, :])
            nc.gpsimd.dma_start(out=wt[64:113, :], in_=wT[:, :])
            for slot in range(2):
                nc.gpsimd.dma_start(
                    out=srct[:, slot * SLAB:(slot + 1) * SLAB],
                    in_=xb[slot, :, :])

            for q in range(NPAIRS):
                i2c = i2c_pool.tile([128, NCOLS], mybir.dt.float16,
                                    tag="i2c")
                for half in range(2):
                    t = 2 * q + half
                    base, slot = _tile_src(t)
                    src = bass.AP(
                        tensor=srct.tensor,
                        offset=srct.offset + base * p_stride + slot * SLAB,
                        ap=[[4 * p_stride, KS], [1, KS], [1, NCOLS]],
                    )
                    b0 = 64 * half
                    nc.gpsimd.dma_start(
                        out=i2c[b0:b0 + KS * KS, :], in_=src)

                ob = ob_pool.tile([128, NCOLS], mybir.dt.float32, tag="ob")
                for j in range(NMM):
                    ps = ps_pool.tile([128, 448], mybir.dt.float32, tag="ps")
                    nc.tensor.matmul(
                        ps[0:OC, :], wt[0:49, :],
                        i2c[0:49, 448 * j: 448 * (j + 1)],
                        start=True, stop=True)
                    nc.tensor.matmul(
                        ps[OC:128, :], wt[64:113, :],
                        i2c[64:113, 448 * j: 448 * (j + 1)],
                        start=True, stop=True)
                    if j % 2 == 0:
                        nc.vector.tensor_copy(
                            ob[:, 448 * j: 448 * (j + 1)], ps[:, :])
                    else:
                        nc.scalar.copy(
                            ob[:, 448 * j: 448 * (j + 1)], ps[:, :])

                for half, eng in ((0, nc.sync), (1, nc.scalar)):
                    t = 2 * q + half
                    img, blk = divmod(t, NBLK)
                    r0 = BLK * blk
                    nrows = min(BLK, OH - r0)
                    eng.dma_start(
                        out=out[img, :, r0: r0 + nrows, :],
                        in_=ob[64 * half: 64 * half + OC, : nrows * OW])

    nc.compile()
    return nc


def _prep_inputs(data, weight):
    d16 = np.ascontiguousarray(data.reshape(B, H, H)).astype(np.float16)
    dpad = np.zeros((B, 256, H), dtype=np.float16)
    dpad[:, :H, :] = d16
    wt = np.ascontiguousarray(
        weight.reshape(OC, KS * KS).T).astype(np.float16)

    in_maps = []
    for c in range(NCORES):
        xb = np.zeros((2, 128, SLAB), dtype=np.float16)
        for t in range(NTILES):
            img, blk = divmod(t, NBLK)
            gimg = c * IPC + img
            base, slot = _tile_src(t)
            for ky in range(KS):
                r0 = BLK * blk + ky
                xb[slot, base + 4 * ky, : SRC_ROWS * H] = \
                    dpad[gimg, r0: r0 + SRC_ROWS, :].ravel()
        in_maps.append({"xb": xb, "wT": wt})
    return in_maps


def kernel(data, weight):
    from concourse.bass_utils import run_bass_kernel_spmd

    if "nc" not in _CACHE:
        _CACHE["nc"] = _build()
    nc = _CACHE["nc"]

    in_maps = _prep_inputs(np.asarray(data), np.asarray(weight))
    res = run_bass_kernel_spmd(nc, in_maps, core_ids=list(range(NCORES)))
    outs = [r["out"] for r in res.results]
    full = np.concatenate(outs, axis=0)  # [32, 64, 218, 224]
    return np.ascontiguousarray(full[:, :, :, :OH]).astype(np.float32)


# revision 6
# speedup vs baseline: 1.7227x; 1.0692x over previous
"""Trainium2 Bass kernel: Conv2d(1->64, k=7, valid) on data [32,1,224,224] f32.

Data-parallel over batch (4 images per core on 8 cores).  Per core:
im2col matmul in fp16 (K=49 taps, M=64 out-channels), PSUM fp32, fp32 out.

Layout/pipeline (per core, 32 row-block "tiles" of 28 output rows):
  - host: fp16 cast; for each tile, SEVEN copies of its 34-row source
    block, copy ky pre-shifted down by ky rows.  Copies of one tile sit at
    partitions base+4*ky (7 distinct AXI ports); even tiles use the lower
    partition half / even ports, odd tiles the upper half / odd ports.
  - im2col: ONE SWDGE DMA per tile (3-dim AP): src dim0 walks the 7 slab
    copies, dim1 the 7 kx shifts (overlapping reads), dim2 a contiguous
    28*224-col run.  dst = [49, 6272] fp16 at partition base 0 (even
    tiles) or 64 (odd tiles).
  - matmul: pairs (even tile, odd tile): lhsT = W^T [49,64] fp16 at row
    base 0/64, out -> psum[0:64]/[64:128] of one bank.  Alternating row
    groups lets LDWEIGHTS overlap in-flight matmuls.
  - copy: psum [128,448] -> ob tile full width, DVE/ACT alternating.
  - out: one DMA per tile [64ch, 28*224 f32]; even tiles on the sync
    HWDGE ring, odd on scalar.  Cols 218..223 are garbage (kx wrap) and
    are sliced off on the host, as are rows >= 218.
"""

import numpy as np

B = 32            # full batch
OC = 64           # out channels
KS = 7            # kernel size
H = 224           # input H=W
OH = 218          # valid output rows/cols
OW = 224          # computed output width (incl 6 garbage cols)
NCORES = 8
IPC = B // NCORES  # images per core

BLK = 28          # output rows per tile
NBLK = 8          # tiles per image
SRC_ROWS = 34     # rows stored per slab copy
SLAB = SRC_ROWS * H + 8   # 7624 fp16 elements per slab
NTILES = IPC * NBLK       # 32 tiles per core
NPAIRS = NTILES // 2
NCOLS = BLK * OW          # 6272 im2col columns per tile
NMM = NCOLS // 448        # 14 matmuls per tile

# slab-group bases: tile t -> 7 slabs at partitions base+4*ky, where
# base = (64 if t odd) + BASES[(t//2) % 8], free slot (t//2) // 8.
BASES = [0, 1, 2, 3, 28, 29, 30, 31]

_CACHE = {}


def _tile_src(t):
    q = t // 2
    base = BASES[q % 8] + (64 if (t % 2) else 0)
    return base, q // 8  # partition base, slot


def _build():
    import concourse.bass as bass
    import concourse.mybir as mybir
    import concourse.tile as tile
    from concourse import bacc

    nc = bacc.Bacc("TRN2", target_bir_lowering=False, debug=False)

    xb = nc.dram_tensor("xb", [2, 128, SLAB], mybir.dt.float16,
                        kind="ExternalInput")
    wT = nc.dram_tensor("wT", [KS * KS, OC], mybir.dt.float16,
                        kind="ExternalInput")
    out = nc.dram_tensor("out", [IPC, OC, OH, OW], mybir.dt.float32,
                         kind="ExternalOutput")

    with tile.TileContext(nc) as tc:
        with (
            tc.tile_pool(name="src", bufs=1) as src_pool,
            tc.tile_pool(name="wp", bufs=1) as w_pool,
            tc.tile_pool(name="i2c", bufs=5) as i2c_pool,
            tc.tile_pool(name="ob", bufs=3) as ob_pool,
            tc.tile_pool(name="ps", bufs=8, space="PSUM") as ps_pool,
        ):
            srct = src_pool.tile([128, 2 * SLAB], mybir.dt.float16)
            wt = w_pool.tile([128, OC], mybir.dt.float16)

            p_stride = srct.ap[0][0]  # partition pitch in elements

            nc.gpsimd.dma_start(out=wt[0:49, :], in_=wT[:, :])
            nc.gpsimd.dma_start(out=wt[64:113, :], in_=wT[:, :])
            for slot in range(2):
                nc.gpsimd.dma_start(
                    out=srct[:, slot * SLAB:(slot + 1) * SLAB],
                    in_=xb[slot, :, :])

            # software-pipelined emission: issue im2col DMAs PREFETCH pairs
            # ahead of the compute stream so the POOL engine's in-order
            # instruction stream never blocks descriptor emission on a
            # downstream dependency.
            PREFETCH = 4
            i2c_tiles = {}

            def issue_i2c(q):
                i2c = i2c_pool.tile([128, NCOLS], mybir.dt.float16,
                                    tag="i2c", name=f"i2c{q}")
                for half in range(2):
                    t = 2 * q + half
                    base, slot = _tile_src(t)
                    src = bass.AP(
                        tensor=srct.tensor,
                        offset=srct.offset + base * p_stride + slot * SLAB,
                        ap=[[4 * p_stride, KS], [1, KS], [1, NCOLS]],
                    )
                    b0 = 64 * half
                    nc.gpsimd.dma_start(
                        out=i2c[b0:b0 + KS * KS, :], in_=src)
                i2c_tiles[q] = i2c

            for q in range(min(PREFETCH, NPAIRS)):
                issue_i2c(q)

            for q in range(NPAIRS):
                if q + PREFETCH < NPAIRS:
                    issue_i2c(q + PREFETCH)
                i2c = i2c_tiles.pop(q)

                ob = ob_pool.tile([128, NCOLS], mybir.dt.float32, tag="ob")
                for j in range(NMM):
                    ps = ps_pool.tile([128, 448], mybir.dt.float32, tag="ps")
                    nc.tensor.matmul(
                        ps[0:OC, :], wt[0:49, :],
                        i2c[0:49, 448 * j: 448 * (j + 1)],
                        start=True, stop=True)
                    nc.tensor.matmul(
                        ps[OC:128, :], wt[64:113, :],
                        i2c[64:113, 448 * j: 448 * (j + 1)],
                        start=True, stop=True)
                    if j % 2 == 0:
                        nc.vector.tensor_copy(
                            ob[:, 448 * j: 448 * (j + 1)], ps[:, :])
                    else:
                        nc.scalar.copy(
                            ob[:, 448 * j: 448 * (j + 1)], ps[:, :])

                for half, eng in ((0, nc.sync), (1, nc.scalar)):
                    t = 2 * q + half
                    img, blk = divmod(t, NBLK)
                    r0 = BLK * blk
                    nrows = min(BLK, OH - r0)
                    eng.dma_start(
                        out=out[img, :, r0: r0 + nrows, :],
                        in_=ob[64 * half: 64 * half + OC, : nrows * OW])

    nc.compile()
    return nc


def _prep_inputs(data, weight):
    d16 = np.ascontiguousarray(data.reshape(B, H, H)).astype(np.float16)
    dpad = np.zeros((B, 256, H), dtype=np.float16)
    dpad[:, :H, :] = d16
    wt = np.ascontiguousarray(
        weight.reshape(OC, KS * KS).T).astype(np.float16)

    in_maps = []
    for c in range(NCORES):
        xb = np.zeros((2, 128, SLAB), dtype=np.float16)
        for t in range(NTILES):
            img, blk = divmod(t, NBLK)
            gimg = c * IPC + img
            base, slot = _tile_src(t)
            for ky in range(KS):
                r0 = BLK * blk + ky
                xb[slot, base + 4 * ky, : SRC_ROWS * H] = \
                    dpad[gimg, r0: r0 + SRC_ROWS, :].ravel()
        in_maps.append({"xb": xb, "wT": wt})
    return in_maps


def kernel(data, weight):
    from concourse.bass_utils import run_bass_kernel_spmd

    if "nc" not in _CACHE:
        _CACHE["nc"] = _build()
    nc = _CACHE["nc"]

    in_maps = _prep_inputs(np.asarray(data), np.asarray(weight))
    res = run_bass_kernel_spmd(nc, in_maps, core_ids=list(range(NCORES)))
    outs = [r["out"] for r in res.results]
    full = np.concatenate(outs, axis=0)  # [32, 64, 218, 224]
    return np.ascontiguousarray(full[:, :, :, :OH]).astype(np.float32)


# revision 8
# speedup vs baseline: 2.1317x; 1.2374x over previous
"""Trainium2 Bass kernel: Conv2d(1->64, k=7, valid) on data [32,1,224,224] f32.

Data-parallel over batch (4 images per core on 8 cores).  Per core:
im2col matmul in fp16 (K=49 taps, M=64 out-channels), PSUM fp32, fp32 out.

Layout/pipeline (per core, 32 row-block "tiles" of 28 output rows):
  - host: fp16 cast; for each tile, SEVEN copies of its 34-row source
    block, copy ky pre-shifted down by ky rows.  Copies of one tile sit at
    partitions base+4*ky (7 distinct AXI ports); even tiles use the lower
    partition half / even ports, odd tiles the upper half / odd ports.
  - im2col: ONE SWDGE DMA per tile (3-dim AP): src dim0 walks the 7 slab
    copies, dim1 the 7 kx shifts (overlapping reads), dim2 a contiguous
    28*224-col run.  dst = [49, 6272] fp16 at partition base 0 (even
    tiles) or 64 (odd tiles).
  - matmul: pairs (even tile, odd tile): lhsT = W^T [49,64] fp16 at row
    base 0/64, out -> psum[0:64]/[64:128] of one bank.  Alternating row
    groups lets LDWEIGHTS overlap in-flight matmuls.
  - copy: psum [128,448] -> ob tile full width, DVE/ACT alternating.
  - out: one DMA per tile [64ch, 28*224 f32]; even tiles on the sync
    HWDGE ring, odd on scalar.  Cols 218..223 are garbage (kx wrap) and
    are sliced off on the host, as are rows >= 218.
"""

import numpy as np

B = 32            # full batch
OC = 64           # out channels
KS = 7            # kernel size
H = 224           # input H=W
OH = 218          # valid output rows/cols
OW = 224          # computed output width (incl 6 garbage cols)
NCORES = 8
IPC = B // NCORES  # images per core

BLK = 28          # output rows per tile
NBLK = 8          # tiles per image
SRC_ROWS = 34     # rows stored per slab copy
SLAB = SRC_ROWS * H + 8   # 7624 fp16 elements per slab
NTILES = IPC * NBLK       # 32 tiles per core
NPAIRS = NTILES // 2
NCOLS = BLK * OW          # 6272 im2col columns per tile
NMM = NCOLS // 448        # 14 matmuls per tile

# slab-group bases: tile t -> 7 slabs at partitions base+4*ky, where
# base = (64 if t odd) + BASES[(t//2) % 8], free slot (t//2) // 8.
BASES = [0, 1, 2, 3, 28, 29, 30, 31]

_CACHE = {}


def _tile_src(t):
    q = t // 2
    base = BASES[q % 8] + (64 if (t % 2) else 0)
    return base, q // 8  # partition base, slot


def _build():
    import concourse.bass as bass
    import concourse.mybir as mybir
    import concourse.tile as tile
    from concourse import bacc

    nc = bacc.Bacc("TRN2", target_bir_lowering=False, debug=False)

    xb = nc.dram_tensor("xb", [2, 128, SLAB], mybir.dt.float16,
                        kind="ExternalInput")
    wT = nc.dram_tensor("wT", [KS * KS, OC], mybir.dt.float16,
                        kind="ExternalInput")
    out = nc.dram_tensor("out", [IPC, OC, OH, OW], mybir.dt.float32,
                         kind="ExternalOutput")

    with tile.TileContext(nc) as tc:
        with (
            tc.tile_pool(name="src", bufs=1) as src_pool,
            tc.tile_pool(name="wp", bufs=1) as w_pool,
            tc.tile_pool(name="i2c", bufs=5) as i2c_pool,
            tc.tile_pool(name="ob", bufs=3) as ob_pool,
            tc.tile_pool(name="ps", bufs=8, space="PSUM") as ps_pool,
        ):
            srct = src_pool.tile([128, 2 * SLAB], mybir.dt.float16)
            wt = w_pool.tile([128, OC], mybir.dt.float16)

            p_stride = srct.ap[0][0]  # partition pitch in elements

            nc.gpsimd.dma_start(out=wt[0:49, :], in_=wT[:, :])
            nc.gpsimd.dma_start(out=wt[64:113, :], in_=wT[:, :])
            for slot in range(2):
                nc.gpsimd.dma_start(
                    out=srct[:, slot * SLAB:(slot + 1) * SLAB],
                    in_=xb[slot, :, :])

            # software-pipelined emission: issue im2col DMAs PREFETCH pairs
            # ahead of the compute stream so the POOL engine's in-order
            # instruction stream never blocks descriptor emission on a
            # downstream dependency.
            PREFETCH = 4
            i2c_tiles = {}

            def issue_i2c(q):
                i2c = i2c_pool.tile([128, NCOLS], mybir.dt.float16,
                                    tag="i2c", name=f"i2c{q}")
                for half in range(2):
                    t = 2 * q + half
                    base, slot = _tile_src(t)
                    src = bass.AP(
                        tensor=srct.tensor,
                        offset=srct.offset + base * p_stride + slot * SLAB,
                        ap=[[4 * p_stride, KS], [1, KS], [1, NCOLS]],
                    )
                    b0 = 64 * half
                    nc.gpsimd.dma_start(
                        out=i2c[b0:b0 + KS * KS, :], in_=src)
                i2c_tiles[q] = i2c

            for q in range(min(PREFETCH, NPAIRS)):
                issue_i2c(q)

            for q in range(NPAIRS):
                if q + PREFETCH < NPAIRS:
                    issue_i2c(q + PREFETCH)
                i2c = i2c_tiles.pop(q)

                ob = ob_pool.tile([128, NCOLS], mybir.dt.float16, tag="ob")
                for j in range(NMM):
                    ps = ps_pool.tile([128, 448], mybir.dt.float32, tag="ps")
                    nc.tensor.matmul(
                        ps[0:OC, :], wt[0:49, :],
                        i2c[0:49, 448 * j: 448 * (j + 1)],
                        start=True, stop=True)
                    nc.tensor.matmul(
                        ps[OC:128, :], wt[64:113, :],
                        i2c[64:113, 448 * j: 448 * (j + 1)],
                        start=True, stop=True)
                    if j % 2 == 0:
                        nc.vector.tensor_copy(
                            ob[:, 448 * j: 448 * (j + 1)], ps[:, :])
                    else:
                        nc.scalar.copy(
                            ob[:, 448 * j: 448 * (j + 1)], ps[:, :])

                # fp16 -> fp32 cast during the store; only SWDGE casts
                for half in range(2):
                    t = 2 * q + half
                    img, blk = divmod(t, NBLK)
                    r0 = BLK * blk
                    nrows = min(BLK, OH - r0)
                    nc.gpsimd.dma_start(
                        out=out[img, :, r0: r0 + nrows, :],
                        in_=ob[64 * half: 64 * half + OC, : nrows * OW])

    nc.compile()
    return nc


def _prep_inputs(data, weight):
    d16 = np.ascontiguousarray(data.reshape(B, H, H)).astype(np.float16)
    dpad = np.zeros((B, 256, H), dtype=np.float16)
    dpad[:, :H, :] = d16
    wt = np.ascontiguousarray(
        weight.reshape(OC, KS * KS).T).astype(np.float16)

    in_maps = []
    for c in range(NCORES):
        xb = np.zeros((2, 128, SLAB), dtype=np.float16)
        for t in range(NTILES):
            img, blk = divmod(t, NBLK)
            gimg = c * IPC + img
            base, slot = _tile_src(t)
            for ky in range(KS):
                r0 = BLK * blk + ky
                xb[slot, base + 4 * ky, : SRC_ROWS * H] = \
                    dpad[gimg, r0: r0 + SRC_ROWS, :].ravel()
        in_maps.append({"xb": xb, "wT": wt})
    return in_maps


def kernel(data, weight):
    from concourse.bass_utils import run_bass_kernel_spmd

    if "nc" not in _CACHE:
        _CACHE["nc"] = _build()
    nc = _CACHE["nc"]

    in_maps = _prep_inputs(np.asarray(data), np.asarray(weight))
    res = run_bass_kernel_spmd(nc, in_maps, core_ids=list(range(NCORES)))
    outs = [r["out"] for r in res.results]
    full = np.concatenate(outs, axis=0)  # [32, 64, 218, 224]
    return np.ascontiguousarray(full[:, :, :, :OH]).astype(np.float32)
